# revision 57
# baseline (speedup 1.0000x reference)
"""DynamicSparseAttention Trainium2 kernel (v4).

Shards B=2 x H=16 across 8 NeuronCores: core c handles batch c//4 and the
4 heads [4*(c%4), 4*(c%4)+4).  Self-contained: all shapes hardcoded.

v4 design (vs v3):
- importance MLP is L-sharded across the 4 cores of each batch: each core
  computes exact 3-term-fp16 scores for its 1024-token slice x all 16 heads,
  then an AllGather over replica groups [[0..3],[4..7]] shares them.  One
  dynamic-offset DMA (partition_id-based) extracts this core's 4-head block.
  Tokens arrive pre-rotated per core so the local slice is rows 0:1024.
- gathered-token transposes use the DMA XBAR (dma transpose, 16x128 tiles)
  instead of PE transposes + DVE copies.
- AV is computed directly in [query, head_dim] orientation (lhsT = expT),
  eliminating the output transposes; normalization reads PSUM directly.
- MLP layer 2 runs in fp32 on the PE (exact; removes the fp16 split).
"""
import numpy as np

import concourse.bass as bass
import concourse.mybir as mybir
import concourse.tile as tile
from concourse import bacc
from concourse.ap import AP
from concourse.bass_utils import run_bass_kernel_spmd

F32 = mybir.dt.float32
F16 = mybir.dt.float16
I16 = mybir.dt.int16
U32 = mybir.dt.uint32
AF = mybir.ActivationFunctionType
OP = mybir.AluOpType

B, L, D = 2, 4096, 1024
H, HD, TOPK = 16, 64, 512
HIDDEN = 256
HPC = 4                # heads per core
COLS = HPC * HD        # 256 output cols per core
NG = 8                 # token groups
GT = 512               # tokens per group
DC = 8                 # 128-row chunks of D
LSL = L // 4           # local MLP token slice
KTH_Q = 1.0 - 510.5 / 4095.0   # k_adj=510 -> out[0,1] = 512th largest
RG = [[0, 1, 2, 3], [4, 5, 6, 7]]


def build_nc():
    nc = bacc.Bacc("TRN2", target_bir_lowering=False, num_devices=8)

    th_t = nc.dram_tensor("th_t", [128, L, DC], F16, kind="ExternalInput")
    tl_t = nc.dram_tensor("tl_t", [128, LSL, DC], F16, kind="ExternalInput")
    tok_lm = nc.dram_tensor("tok_lm", [L, D], F16, kind="ExternalInput")
    iota_r = nc.dram_tensor("iota_r", [128, 256], F32, kind="ExternalInput")
    wq = nc.dram_tensor("wq", [D, COLS], F16, kind="ExternalInput")
    wkv = nc.dram_tensor("wkv", [D, 2 * COLS], F16, kind="ExternalInput")
    bqt = nc.dram_tensor("bqt", [128, 2], F32, kind="ExternalInput")
    bvt = nc.dram_tensor("bvt", [128, HPC], F32, kind="ExternalInput")
    wi1h = nc.dram_tensor("wi1h", [D, HIDDEN], F16, kind="ExternalInput")
    wi1l = nc.dram_tensor("wi1l", [D, HIDDEN], F16, kind="ExternalInput")
    wi2f = nc.dram_tensor("wi2f", [HIDDEN, H], F32, kind="ExternalInput")
    bi1t = nc.dram_tensor("bi1t", [128, 2], F32, kind="ExternalInput")
    bi2 = nc.dram_tensor("bi2", [H, 1], F32, kind="ExternalInput")

    gat_in = nc.dram_tensor("gat_in", [H, LSL], F32, kind="Internal")
    gat_out = nc.dram_tensor("gat_out", [4, H, LSL], F32, kind="Internal")

    out = nc.dram_tensor("out", [L, COLS], F16, kind="ExternalOutput")
    dbg_scores = nc.dram_tensor("dbg_scores", [HPC, L], F32, kind="ExternalOutput")

    # ---- constants ----
    ident16_d = nc.inline_tensor(np.eye(128, dtype=np.float16), name="ident16")
    r16_np = (np.arange(16)[:, None] == (np.arange(128)[None, :] % 16)).astype(np.float32)
    r16_d = nc.inline_tensor(r16_np, name="r16")
    b4_np = (np.arange(128)[None, None, :] // 32
             == np.arange(4)[None, :, None]).astype(np.float32)
    b4_d = nc.inline_tensor(b4_np, name="b4")  # [1, 4, 128]

    with tile.TileContext(nc) as tc:
        with (
            tc.tile_pool(name="persist", bufs=1) as pp_,
            tc.tile_pool(name="ring", bufs=2) as pr,
            tc.tile_pool(name="small", bufs=2) as psm,
            tc.tile_pool(name="psX", bufs=3, space="PSUM") as psX,
            tc.tile_pool(name="psT", bufs=2, space="PSUM") as psT,
        ):
            # ---------- persistent ----------
            tokh = pp_.tile([128, L, DC], F16)       # 64KB/part
            qT = pp_.tile([128, 2, L], F16)          # 16KB/part
            sc2 = pp_.tile([128, 256], F32)
            # rows 16:32 of each 32-row head block are never DMA'd; zero them
            # once so the is_ge mask reads defined data (iota masks them out).
            nc.gpsimd.memset(sc2, 0.0)
            sck = pp_.tile([128, HPC, 32], F32)
            obuf = pp_.tile([128, 32, COLS], F16)    # 16KB/part

            # ---------- input DMAs, ordered for the MLP critical path ------
            # sync (SP) queue: MLP tokens, then (blocking) imp stages
            # scalar (Act) queue: weights, then the q-proj token stream
            # PE warm-up: the cost model clocks the PE up only after ~3us of
            # activity (LOW->MID->FULL). Run a throwaway accumulation chain on
            # the identity tile (first DMA, 32KB) so the ramp completes while
            # the real inputs are still streaming in.
            ident16 = pp_.tile([128, 128], F16)
            nc.sync.dma_start(ident16, ident16_d[:, :])
            warm = psT.tile([128, 128], F32, tag="tp", name="warm")
            NWARM = 90
            for i in range(NWARM):
                nc.tensor.matmul(warm, ident16, ident16,
                                 start=(i == 0), stop=(i == NWARM - 1))
            # preload the Act function tables (Gelu/Exp/Identity) while the
            # engine is idle so no 1.3us table load lands on the critical path
            actwarm = psm.tile([1, 2], F16, tag="actwarm")
            nc.scalar.activation(actwarm[0:1, 0:1], ident16[0:1, 0:1],
                                 AF.Gelu, scale=1.0)
            nc.scalar.activation(actwarm[0:1, 1:2], ident16[0:1, 0:1],
                                 AF.Exp, scale=1.0)
            nc.scalar.copy(actwarm[0:1, 0:1], ident16[0:1, 0:1])

            tokls = []
            wi1hs = pp_.tile([128, DC, HIDDEN], F16)
            wi1ls = pp_.tile([128, DC, HIDDEN], F16)
            for g in range(2):
                sl = slice(g * GT, (g + 1) * GT)
                nc.sync.dma_start(tokh[:, sl, :], th_t[:, sl, :])
                w, wsrc = (wi1hs, wi1h) if g == 0 else (wi1ls, wi1l)
                nc.scalar.dma_start(w, wsrc.rearrange("(c p) n -> p c n", p=128))
                tokl = pr.tile([128, GT, DC], F16, tag="tokl", name="tokl")
                nc.sync.dma_start(tokl, tl_t[:, sl, :])
                tokls.append(tokl)
            wi2fs = pp_.tile([128, 2, H], F32)
            nc.scalar.dma_start(wi2fs, wi2f.rearrange("(c p) n -> p c n", p=128))
            bi1s = pp_.tile([128, 2], F32)
            nc.sync.dma_start(bi1s, bi1t[:, :])
            bi2s = pp_.tile([H, 1], F32)
            nc.sync.dma_start(bi2s, bi2[:, :])
            bqs = pp_.tile([128, 2], F32)
            nc.sync.dma_start(bqs, bqt[:, :])
            bvs = pp_.tile([128, HPC], F32)
            nc.sync.dma_start(bvs, bvt[:, :])
            iota1 = pp_.tile([128, 256], F32)
            nc.sync.dma_start(iota1, iota_r[:, :])
            r16 = pp_.tile([16, 128], F32)
            nc.sync.dma_start(r16, r16_d[:, :])
            b4 = pp_.tile([1, 4, 128], F32)
            nc.sync.dma_start(b4, b4_d[:, :, :])
            wqs = pp_.tile([128, DC, COLS], F16)
            nc.scalar.dma_start(wqs, wq.rearrange("(c p) n -> p c n", p=128))
            wkvs = pp_.tile([128, DC, 2 * COLS], F16)
            nc.scalar.dma_start(wkvs, wkv.rearrange("(c p) n -> p c n", p=128))

            # ---------- phase A: local-slice importance MLP ----------
            hpss = []
            for g in range(2):
                sl = slice(g * GT, (g + 1) * GT)
                hps = psX.tile([128, 2, GT], F32, tag="psX")
                hpss.append(hps)
                # term-outer order: the first 16 matmuls only need th + wi1h,
                # so compute starts before tl/wi1l arrive.
                terms = ((wi1hs, tokh[:, sl, :]), (wi1hs, tokls[g]),
                         (wi1ls, tokh[:, sl, :]))
                for ti, (w, a) in enumerate(terms):
                    for ht in range(2):
                        hsl = slice(ht * 128, (ht + 1) * 128)
                        for j in range(DC):
                            nc.tensor.matmul(
                                hps[:, ht, :], w[:, j, hsl], a[:, :, j],
                                start=(ti == 0 and j == 0),
                                stop=(ti == 2 and j == DC - 1))
            for g in range(2):
                sl = slice(g * GT, (g + 1) * GT)
                gtmp = pr.tile([128, 2, GT], F32, tag="gtmp", name="gtmp")
                for ht in range(2):
                    nc.scalar.activation(gtmp[:, ht, :], hpss[g][:, ht, :],
                                         AF.Gelu, bias=bi1s[:, ht:ht + 1],
                                         scale=1.0)
                ips = psX.tile([H, GT], F32, tag="psX", name="ips")
                for kc in range(2):
                    nc.tensor.matmul(ips, wi2fs[:, kc, :], gtmp[:, kc, :],
                                     start=(kc == 0), stop=(kc == 1))
                imp_c = psm.tile([H, GT], F32, tag="imp_c")
                nc.vector.tensor_scalar_add(imp_c, ips, bi2s)
                nc.sync.dma_start(gat_in[:, sl], imp_c)

            # ---------- score all-gather across the 4 cores of this batch ----
            nc.gpsimd.collective_compute(
                "AllGather", mybir.AluOpType.bypass, replica_groups=RG,
                ins=[gat_in[:, :].opt()], outs=[gat_out[:, :, :].opt()])

            # scheduler fence: without it the (dep-free) token-stream DMAs
            # below get scheduled ahead of the imp DMAs and delay the
            # collective behind 18us of streaming at the DMA-engine FIFO.
            tc.no_sync_barrier()

            # q-proj token stream: emitted after the imp stages so its DMA
            # transfers queue behind them at the (FIFO) DMA engines.
            for g in range(2, NG):
                sl = slice(g * GT, (g + 1) * GT)
                nc.sync.dma_start(tokh[:, sl, :], th_t[:, sl, :])

            # dynamic offsets (elements) of this core's 4-head block
            pid_sp = nc.sync.partition_id()
            hoff_sp = (pid_sp % 4) * (HPC * LSL)
            pid_act = nc.scalar.partition_id()
            hoff_act = (pid_act % 4) * (HPC * LSL)
            pid = nc.gpsimd.partition_id()
            hoff = (pid % 4) * (HPC * LSL)
            gt_h = gat_out[:, :, :].tensor

            # score loads for selection, spread across both HWDGE queues
            # sck[r*32+pp, h, c] = gat_out[r, 4hg+h, pp*32+c] (order-free)
            for h in range(HPC):
                nc.sync.dma_start(sck[:, h, :], AP(
                    tensor=gt_h, offset=hoff_sp + h * LSL,
                    ap=[[H * LSL, 4], [32, 32], [1, 32]]))
            # sc2[32h+pp, ff] = score[h, pp*256+ff]
            for h in range(HPC):
                nc.scalar.dma_start(sc2[32 * h:32 * h + 16, :], AP(
                    tensor=gt_h, offset=hoff_act + h * LSL,
                    ap=[[H * LSL, 4], [256, 4], [1, 256]]))

            # ---------- selection state ----------
            v4 = pp_.tile([1, HPC, 2], F32)
            sel = pp_.tile([128, 256], F32)
            idx4 = pp_.tile([16, HPC, 32], F32)
            nfound = pp_.tile([16, HPC], U32)
            idx16 = pp_.tile([128, HPC, 32], I16)
            kTs_t = pp_.tile([128, HPC, TOPK], F16)
            vselA = pp_.tile([128, HPC, 4, HD + 1], F16)
            nc.vector.memset(vselA[:, :, :, HD:HD + 1], 1.0)

            # ---------- pass B: q projection ----------
            for g in range(NG):
                sl = slice(g * GT, (g + 1) * GT)
                qp = psX.tile([128, 2, GT], F32, tag="psX", name="qp")
                for p2 in range(2):
                    for j in range(DC):
                        nc.tensor.matmul(
                            qp[:, p2, :], wqs[:, j, p2 * 128:(p2 + 1) * 128],
                            tokh[:, sl, j], start=(j == 0), stop=(j == DC - 1))
                    nc.vector.tensor_scalar_add(qT[:, p2, sl], qp[:, p2, :],
                                                bqs[:, p2:p2 + 1])

            def sel_phase():
                # engine-major emission: per-engine queues are in-order, so
                # grouping by engine keeps head h's chain from blocking
                # head h+1's independent instructions.
                for h in range(HPC):
                    nc.gpsimd.kth_largest(v4[0:1, h, :], sck[:, h, :],
                                          n_per_lane=32, k=510, quantile=KTH_Q)
                thps = []
                for h in range(HPC):
                    thp = psT.tile([128, 1], F32, tag="tp", name="thp")
                    nc.tensor.matmul(thp, b4[0:1, h, :], v4[0:1, h, 1:2],
                                     start=True, stop=True)
                    thps.append(thp)
                for h in range(HPC):
                    ssl = slice(32 * h, 32 * h + 16)
                    nc.vector.tensor_scalar(sel[ssl, :], sc2[ssl, :],
                                            thps[h][ssl, :], None, op0=OP.is_ge)
                    nc.vector.tensor_mul(sel[ssl, :], sel[ssl, :],
                                         iota1[ssl, :])
                    nc.vector.tensor_scalar_sub(sel[ssl, :], sel[ssl, :], 1.0)
                selhs = []
                for h in range(HPC):
                    selh = psm.tile([16, 256], F32, tag="selh", bufs=4)
                    # Act copy (partition-shifting) keeps Pool free for the
                    # kth/sparse_gather chain
                    nc.scalar.copy(selh, sel[32 * h:32 * h + 16, :])
                    selhs.append(selh)
                rps = {}

                def sel_tail(h):
                    nc.gpsimd.sparse_gather(
                        idx4[:, h, :], selhs[h], num_found=nfound[0:1, h:h + 1])
                    rp = psT.tile([128, 32], F32, tag="tp", name="rp")
                    nc.tensor.matmul(rp, r16, idx4[:, h, :],
                                     start=True, stop=True)
                    nc.vector.tensor_copy(idx16[:, h, :], rp)

                # head 0's tail first so its gather issues ahead of the rest
                sel_tail(0)
                return sel_tail

            def head_gather(h):
                toksel = pr.tile([128, 4, D], F16, tag="toksel",
                                 name="toksel", bufs=2)
                nc.gpsimd.dma_gather(
                    toksel, tok_lm[:, :], idx16[:, h, :], num_idxs=TOPK,
                    num_idxs_reg=TOPK, elem_size=D, elem_step=D)
                return toksel

            def head_xbar(h, toksel, jr=range(DC), tokB=None):
                # PE transposes: tokB[p, j, kt*128+k] = toksel[k, kt, j*128+p]
                if tokB is None:
                    tokB = pr.tile([128, DC, TOPK], F16, tag="tokB",
                                   name="tokB", bufs=2)
                for j in jr:
                    for half in range(2):
                        ttp = psT.tile([128, 2, 128], F16, tag="tp", name="ttp")
                        for kk in range(2):
                            kt = half * 2 + kk
                            nc.tensor.transpose(
                                ttp[:, kk, :],
                                toksel[:, kt, j * 128:(j + 1) * 128], ident16)
                        nc.vector.tensor_copy(
                            tokB[:, j, 256 * half:256 * (half + 1)],
                            ttp.rearrange("p a b -> p (a b)"))
                return tokB

            def head_proj_kv_a(h, tokB):
                kvp = psX.tile([128, TOPK], F32, tag="psX", name="kvp")
                for j in range(4):
                    nc.tensor.matmul(kvp, wkvs[:, j, 128 * h:128 * (h + 1)],
                                     tokB[:, j, :], start=(j == 0), stop=False)
                return kvp

            def head_proj_kv_b(h, kvp, tokB):
                h2 = h % 2
                # fused kv projection: host orders the wkv block so k lands on
                # rows [64h2, 64h2+64) (matching qT/kTs_t) and v on the other
                # 64 rows — all PSUM drains keep partition alignment for DVE.
                ksl = slice(64 * h2, 64 * h2 + 64)
                vsl = slice(64 - 64 * h2, 128 - 64 * h2)
                for j in range(4, DC):
                    nc.tensor.matmul(kvp, wkvs[:, j, 128 * h:128 * (h + 1)],
                                     tokB[:, j, :],
                                     start=False, stop=(j == DC - 1))
                nc.vector.tensor_copy(kTs_t[ksl, h, :], kvp[ksl, :])
                vTb = psm.tile([128, TOPK], F16, tag="vTb", bufs=2)
                nc.vector.tensor_scalar_add(vTb[vsl, :], kvp[vsl, :],
                                            bvs[vsl, h:h + 1])
                return vTb

            def head_proj_mm(h, tokB):
                kvp = head_proj_kv_a(h, tokB)
                return head_proj_kv_b(h, kvp, tokB)

            def head_proj_vsel(h, vTb):
                h2 = h % 2
                vsl = slice(64 - 64 * h2, 128 - 64 * h2)
                # PE transposes: vselA[p, h, kt, d] = v[d, kt*128+p]
                for kt in range(4):
                    vtp = psT.tile([128, HD], F16, tag="tp", name="vtp")
                    nc.tensor.transpose(vtp, vTb[vsl, kt * 128:(kt + 1) * 128],
                                        ident16[vsl, vsl])
                    nc.vector.tensor_copy(vselA[:, h, kt, 0:HD], vtp)

            def attn_logits(h, qc):
                p2, h2 = h // 2, h % 2
                expT = [None, None]
                for half in range(2):
                    lp = psX.tile([128, 2, GT], F32, tag="psX", name="lp")
                    for kk in range(2):
                        kt = half * 2 + kk
                        nc.tensor.matmul(
                            lp[:, kk, :],
                            kTs_t[64 * h2:64 * h2 + 64, h,
                                  kt * 128:(kt + 1) * 128],
                            qT[64 * h2:64 * h2 + 64, p2,
                               qc * 512:(qc + 1) * 512],
                            start=True, stop=True)
                    expT[half] = pr.tile([128, 2, GT], F16, tag="expT",
                                         name="expT", bufs=4)
                    nc.scalar.activation(
                        expT[half].rearrange("p a b -> p (a b)"),
                        lp.rearrange("p a b -> p (a b)"),
                        AF.Exp, scale=0.125)
                return expT

            def attn_av(h, qc, expT):
                avp = psT.tile([128, 4, HD + 1], F32, tag="tp", name="avp")
                for qs in range(4):
                    for kt in range(4):
                        nc.tensor.matmul(
                            avp[:, qs, :],
                            expT[kt // 2][:, kt % 2, qs * 128:(qs + 1) * 128],
                            vselA[:, h, kt, :],
                            start=(kt == 0), stop=(kt == 3))
                rcp = psm.tile([128, 4], F32, tag="rcp")
                nc.vector.reciprocal(
                    rcp, avp[:, :, HD:HD + 1].rearrange("p a b -> p (a b)"))
                for qs in range(4):
                    qt = qc * 4 + qs
                    nc.vector.tensor_scalar_mul(
                        obuf[:, qt, HD * h:HD * (h + 1)], avp[:, qs, 0:HD],
                        rcp[:, qs:qs + 1])

            sel_tail = sel_phase()
            toksel0 = head_gather(0)
            for h in range(1, HPC):
                sel_tail(h)
            tokB0 = head_xbar(0, toksel0)
            vTb0 = head_proj_mm(0, tokB0)
            # head 0's first logits go out before the v-side prep so the Act
            # engine's exp backbone starts as early as possible
            expT_cur = attn_logits(0, 0)
            head_proj_vsel(0, vTb0)

            # attention, software-pipelined one qc ahead: the next qc's
            # logits+exp are emitted before this qc's AV so exp runs
            # back-to-back on Act (the attention-phase bottleneck).
            tksl, tkb, tvtb = {}, {}, {}
            for h in range(HPC):
                for qc in range(8):
                    # next qc's logits first: the Act exp backbone must never
                    # wait behind prep work in the PE stream
                    if qc < 7:
                        expT_next = attn_logits(h, qc + 1)
                    elif h + 1 < HPC:
                        expT_next = attn_logits(h + 1, 0)
                    else:
                        expT_next = None
                    if h + 1 < HPC:
                        if qc == 0:
                            tksl[h + 1] = head_gather(h + 1)
                        elif qc <= 4:
                            jr = range(2 * (qc - 1), 2 * qc)
                            tkb[h + 1] = head_xbar(
                                h + 1, tksl[h + 1], jr=jr,
                                tokB=tkb.get(h + 1) if qc > 1 else None)
                        elif qc == 5:
                            tvtb[h + 1] = head_proj_mm(h + 1, tkb[h + 1])
                        elif qc == 6:
                            head_proj_vsel(h + 1, tvtb[h + 1])
                    attn_av(h, qc, expT_cur)
                    expT_cur = expT_next
                    if h == HPC - 1:
                        # sync queue: idle during attention, and splitting the
                        # last block halves the post-compute drain.
                        osl = out[qc * 512:(qc + 1) * 512, :].rearrange(
                            "(q p) n -> p q n", p=128)
                        if qc < 7:
                            nc.sync.dma_start(osl, obuf[:, qc * 4:(qc + 1) * 4, :])
                        else:
                            nc.sync.dma_start(osl[:, 0:2, :],
                                              obuf[:, qc * 4:qc * 4 + 2, :])
                            nc.sync.dma_start(osl[:, 2:4, :],
                                              obuf[:, qc * 4 + 2:qc * 4 + 4, :])

            # dbg_scores[h, r*1024 + t] = gat_out[r, 4hg+h, t] — test-harness
            # output only; emitted last so it never touches the critical path.
            nc.gpsimd.dma_start(
                dbg_scores[:, :].rearrange("h (r t) -> h r t", r=4),
                AP(tensor=gt_h, offset=hoff,
                   ap=[[LSL, HPC], [H * LSL, 4], [1, LSL]]))

    nc.compile()
    return nc


_NC = None


def _get_nc():
    global _NC
    if _NC is None:
        _NC = build_nc()
    return _NC


def make_in_maps(**inputs):
    t = {k: np.ascontiguousarray(np.asarray(v, dtype=np.float32))
         for k, v in inputs.items()}
    wi1h = t["Wi1"].astype(np.float16)
    wi1l = (t["Wi1"] - wi1h.astype(np.float32)).astype(np.float16)
    in_maps = []
    for c in range(8):
        b, hg = c // 4, c % 4
        cs = COLS * hg
        wkv_c = np.empty((D, 2 * COLS), dtype=np.float16)
        for hh in range(HPC):
            # k on rows [64*(hh%2), +64), v on the complementary 64 rows
            ko, vo = (0, 64) if hh % 2 == 0 else (64, 0)
            wkv_c[:, 128 * hh + ko:128 * hh + ko + 64] = \
                t["Wk"][:, cs + 64 * hh:cs + 64 * hh + 64]
            wkv_c[:, 128 * hh + vo:128 * hh + vo + 64] = \
                t["Wv"][:, cs + 64 * hh:cs + 64 * hh + 64]
        tokT = np.ascontiguousarray(t["tokens"][b].T)   # [D, L]
        th = tokT.astype(np.float16)
        # rotate so this core's MLP slice sits at rows 0:1024
        th_rot = np.roll(th, -LSL * hg, axis=1)
        sl = slice(LSL * hg, LSL * (hg + 1))
        tl = (tokT[:, sl] - th[:, sl].astype(np.float32)).astype(np.float16)
        # gather-friendly [128, L, 8]: th_g[p, l, c] = th[c*128+p, l]
        th_rot = np.ascontiguousarray(
            th_rot.reshape(DC, 128, L).transpose(1, 2, 0))
        tl = np.ascontiguousarray(tl.reshape(DC, 128, LSL).transpose(1, 2, 0))
        pp = np.arange(128) % 32
        iota_np = np.where(
            pp[:, None] < 16,
            (pp[:, None] * 256 + np.arange(256)[None, :]).astype(np.float64)
            + 1.0,
            -1e9).astype(np.float32)
        in_maps.append({
            "th_t": th_rot,
            "tl_t": tl,
            "tok_lm": np.ascontiguousarray(t["tokens"][b].astype(np.float16)),
            "iota_r": np.ascontiguousarray(iota_np),
            "wq": np.ascontiguousarray(t["Wq"][:, cs:cs + COLS]).astype(np.float16),
            "wkv": wkv_c,
            "bqt": np.ascontiguousarray(t["bq"][cs:cs + COLS].reshape(2, 128).T),
            "bvt": np.ascontiguousarray(
                np.tile(t["bv"][cs:cs + COLS].reshape(4, 64).T, (2, 1))),
            "wi1h": wi1h,
            "wi1l": wi1l,
            "wi2f": t["Wi2"],
            "bi1t": np.ascontiguousarray(t["bi1"].reshape(2, 128).T),
            "bi2": np.ascontiguousarray(t["bi2"].reshape(H, 1)),
        })
    return in_maps


def kernel(**inputs) -> np.ndarray:
    nc = _get_nc()
    in_maps = make_in_maps(**inputs)
    res = run_bass_kernel_spmd(nc, in_maps, core_ids=list(range(8)))
    out = np.empty((B, L, D), dtype=np.float32)
    for c in range(8):
        b, hg = c // 4, c % 4
        o = res.results[c]["out"].astype(np.float32)
        out[b, :, COLS * hg:COLS * (hg + 1)] = np.roll(o, LSL * hg, axis=0)
    return out


# revision 58
# speedup vs baseline: 1.0124x; 1.0124x over previous
"""DynamicSparseAttention Trainium2 kernel (v4).

Shards B=2 x H=16 across 8 NeuronCores: core c handles batch c//4 and the
4 heads [4*(c%4), 4*(c%4)+4).  Self-contained: all shapes hardcoded.

v4 design (vs v3):
- importance MLP is L-sharded across the 4 cores of each batch: each core
  computes exact 3-term-fp16 scores for its 1024-token slice x all 16 heads,
  then an AllGather over replica groups [[0..3],[4..7]] shares them.  One
  dynamic-offset DMA (partition_id-based) extracts this core's 4-head block.
  Tokens arrive pre-rotated per core so the local slice is rows 0:1024.
- gathered-token transposes use the DMA XBAR (dma transpose, 16x128 tiles)
  instead of PE transposes + DVE copies.
- AV is computed directly in [query, head_dim] orientation (lhsT = expT),
  eliminating the output transposes; normalization reads PSUM directly.
- MLP layer 2 runs in fp32 on the PE (exact; removes the fp16 split).
"""
import numpy as np

import concourse.bass as bass
import concourse.mybir as mybir
import concourse.tile as tile
from concourse import bacc
from concourse.ap import AP
from concourse.bass_utils import run_bass_kernel_spmd

F32 = mybir.dt.float32
F16 = mybir.dt.float16
I16 = mybir.dt.int16
U32 = mybir.dt.uint32
AF = mybir.ActivationFunctionType
OP = mybir.AluOpType

B, L, D = 2, 4096, 1024
H, HD, TOPK = 16, 64, 512
HIDDEN = 256
HPC = 4                # heads per core
COLS = HPC * HD        # 256 output cols per core
NG = 8                 # token groups
GT = 512               # tokens per group
DC = 8                 # 128-row chunks of D
LSL = L // 4           # local MLP token slice
KTH_Q = 1.0 - 510.5 / 4095.0   # k_adj=510 -> out[0,1] = 512th largest
RG = [[0, 1, 2, 3], [4, 5, 6, 7]]


def build_nc():
    nc = bacc.Bacc("TRN2", target_bir_lowering=False, num_devices=8)

    th_t = nc.dram_tensor("th_t", [128, L, DC], F16, kind="ExternalInput")
    tl_t = nc.dram_tensor("tl_t", [128, LSL, DC], F16, kind="ExternalInput")
    tok_lm = nc.dram_tensor("tok_lm", [L, D], F16, kind="ExternalInput")
    iota_r = nc.dram_tensor("iota_r", [128, 256], F32, kind="ExternalInput")
    wq = nc.dram_tensor("wq", [D, COLS], F16, kind="ExternalInput")
    wkv = nc.dram_tensor("wkv", [D, 2 * COLS], F16, kind="ExternalInput")
    bqt = nc.dram_tensor("bqt", [128, 2], F32, kind="ExternalInput")
    bvt = nc.dram_tensor("bvt", [128, HPC], F32, kind="ExternalInput")
    wi1h = nc.dram_tensor("wi1h", [D, HIDDEN], F16, kind="ExternalInput")
    wi1l = nc.dram_tensor("wi1l", [D, HIDDEN], F16, kind="ExternalInput")
    wi2f = nc.dram_tensor("wi2f", [HIDDEN, H], F32, kind="ExternalInput")
    bi1t = nc.dram_tensor("bi1t", [128, 2], F32, kind="ExternalInput")
    bi2 = nc.dram_tensor("bi2", [H, 1], F32, kind="ExternalInput")

    gat_in = nc.dram_tensor("gat_in", [H, LSL], F32, kind="Internal")
    gat_out = nc.dram_tensor("gat_out", [4, H, LSL], F32, kind="Internal")

    out = nc.dram_tensor("out", [L, COLS], F16, kind="ExternalOutput")
    dbg_scores = nc.dram_tensor("dbg_scores", [HPC, L], F32, kind="ExternalOutput")

    # ---- constants ----
    ident16_d = nc.inline_tensor(np.eye(128, dtype=np.float16), name="ident16")
    r16_np = (np.arange(16)[:, None] == (np.arange(128)[None, :] % 16)).astype(np.float32)
    r16_d = nc.inline_tensor(r16_np, name="r16")
    b4_np = (np.arange(128)[None, None, :] // 32
             == np.arange(4)[None, :, None]).astype(np.float32)
    b4_d = nc.inline_tensor(b4_np, name="b4")  # [1, 4, 128]

    with tile.TileContext(nc) as tc:
        with (
            tc.tile_pool(name="persist", bufs=1) as pp_,
            tc.tile_pool(name="ring", bufs=2) as pr,
            tc.tile_pool(name="small", bufs=2) as psm,
            tc.tile_pool(name="psX", bufs=3, space="PSUM") as psX,
            tc.tile_pool(name="psT", bufs=2, space="PSUM") as psT,
        ):
            # ---------- persistent ----------
            tokh = pp_.tile([128, L, DC], F16)       # 64KB/part
            qT = pp_.tile([128, 2, L], F16)          # 16KB/part
            sc2 = pp_.tile([128, 256], F32)
            # rows 16:32 of each 32-row head block are never DMA'd; zero them
            # once so the is_ge mask reads defined data (iota masks them out).
            nc.gpsimd.memset(sc2, 0.0)
            sck = pp_.tile([128, HPC, 32], F32)
            obuf = pp_.tile([128, 32, COLS], F16)    # 16KB/part

            # ---------- input DMAs, ordered for the MLP critical path ------
            # sync (SP) queue: MLP tokens, then (blocking) imp stages
            # scalar (Act) queue: weights, then the q-proj token stream
            # PE warm-up: the cost model clocks the PE up only after ~3us of
            # activity (LOW->MID->FULL). Run a throwaway accumulation chain on
            # the identity tile (first DMA, 32KB) so the ramp completes while
            # the real inputs are still streaming in.
            ident16 = pp_.tile([128, 128], F16)
            nc.sync.dma_start(ident16, ident16_d[:, :])
            warm = psT.tile([128, 128], F32, tag="tp", name="warm")
            NWARM = 90
            for i in range(NWARM):
                nc.tensor.matmul(warm, ident16, ident16,
                                 start=(i == 0), stop=(i == NWARM - 1))
            # preload the Act function tables (Gelu/Exp/Identity) while the
            # engine is idle so no 1.3us table load lands on the critical path
            actwarm = psm.tile([1, 2], F16, tag="actwarm")
            nc.scalar.activation(actwarm[0:1, 0:1], ident16[0:1, 0:1],
                                 AF.Gelu, scale=1.0)
            nc.scalar.activation(actwarm[0:1, 1:2], ident16[0:1, 0:1],
                                 AF.Exp, scale=1.0)
            nc.scalar.copy(actwarm[0:1, 0:1], ident16[0:1, 0:1])

            tokls = []
            wi1hs = pp_.tile([128, DC, HIDDEN], F16)
            wi1ls = pp_.tile([128, DC, HIDDEN], F16)
            for g in range(2):
                sl = slice(g * GT, (g + 1) * GT)
                nc.sync.dma_start(tokh[:, sl, :], th_t[:, sl, :])
                w, wsrc = (wi1hs, wi1h) if g == 0 else (wi1ls, wi1l)
                nc.scalar.dma_start(w, wsrc.rearrange("(c p) n -> p c n", p=128))
                tokl = pr.tile([128, GT, DC], F16, tag="tokl", name="tokl")
                nc.sync.dma_start(tokl, tl_t[:, sl, :])
                tokls.append(tokl)
            wi2fs = pp_.tile([128, 2, H], F32)
            nc.scalar.dma_start(wi2fs, wi2f.rearrange("(c p) n -> p c n", p=128))
            bi1s = pp_.tile([128, 2], F32)
            nc.sync.dma_start(bi1s, bi1t[:, :])
            bi2s = pp_.tile([H, 1], F32)
            nc.sync.dma_start(bi2s, bi2[:, :])
            bqs = pp_.tile([128, 2], F32)
            nc.sync.dma_start(bqs, bqt[:, :])
            bvs = pp_.tile([128, HPC], F32)
            nc.sync.dma_start(bvs, bvt[:, :])
            iota1 = pp_.tile([128, 256], F32)
            nc.sync.dma_start(iota1, iota_r[:, :])
            r16 = pp_.tile([16, 128], F32)
            nc.sync.dma_start(r16, r16_d[:, :])
            b4 = pp_.tile([1, 4, 128], F32)
            nc.sync.dma_start(b4, b4_d[:, :, :])
            wqs = pp_.tile([128, DC, COLS], F16)
            nc.scalar.dma_start(wqs, wq.rearrange("(c p) n -> p c n", p=128))
            wkvs = pp_.tile([128, DC, 2 * COLS], F16)
            nc.scalar.dma_start(wkvs, wkv.rearrange("(c p) n -> p c n", p=128))

            # ---------- phase A: local-slice importance MLP ----------
            hpss = []
            for g in range(2):
                sl = slice(g * GT, (g + 1) * GT)
                hps = psX.tile([128, 2, GT], F32, tag="psX")
                hpss.append(hps)
                # term-outer order: the first 16 matmuls only need th + wi1h,
                # so compute starts before tl/wi1l arrive.
                terms = ((wi1hs, tokh[:, sl, :]), (wi1hs, tokls[g]),
                         (wi1ls, tokh[:, sl, :]))
                for ti, (w, a) in enumerate(terms):
                    for ht in range(2):
                        hsl = slice(ht * 128, (ht + 1) * 128)
                        for j in range(DC):
                            nc.tensor.matmul(
                                hps[:, ht, :], w[:, j, hsl], a[:, :, j],
                                start=(ti == 0 and j == 0),
                                stop=(ti == 2 and j == DC - 1))
            for g in range(2):
                sl = slice(g * GT, (g + 1) * GT)
                gtmp = pr.tile([128, 2, GT], F32, tag="gtmp", name="gtmp")
                for ht in range(2):
                    nc.scalar.activation(gtmp[:, ht, :], hpss[g][:, ht, :],
                                         AF.Gelu, bias=bi1s[:, ht:ht + 1],
                                         scale=1.0)
                ips = psX.tile([H, GT], F32, tag="psX", name="ips")
                for kc in range(2):
                    nc.tensor.matmul(ips, wi2fs[:, kc, :], gtmp[:, kc, :],
                                     start=(kc == 0), stop=(kc == 1))
                imp_c = psm.tile([H, GT], F32, tag="imp_c")
                nc.vector.tensor_scalar_add(imp_c, ips, bi2s)
                nc.sync.dma_start(gat_in[:, sl], imp_c)

            # ---------- score all-gather across the 4 cores of this batch ----
            nc.gpsimd.collective_compute(
                "AllGather", mybir.AluOpType.bypass, replica_groups=RG,
                ins=[gat_in[:, :].opt()], outs=[gat_out[:, :, :].opt()])

            # scheduler fence: without it the (dep-free) token-stream DMAs
            # below get scheduled ahead of the imp DMAs and delay the
            # collective behind 18us of streaming at the DMA-engine FIFO.
            tc.no_sync_barrier()

            # q-proj token stream: emitted after the imp stages so its DMA
            # transfers queue behind them at the (FIFO) DMA engines.
            for g in range(2, NG):
                sl = slice(g * GT, (g + 1) * GT)
                nc.sync.dma_start(tokh[:, sl, :], th_t[:, sl, :])

            # dynamic offsets (elements) of this core's 4-head block
            pid_sp = nc.sync.partition_id()
            hoff_sp = (pid_sp % 4) * (HPC * LSL)
            pid_act = nc.scalar.partition_id()
            hoff_act = (pid_act % 4) * (HPC * LSL)
            pid = nc.gpsimd.partition_id()
            hoff = (pid % 4) * (HPC * LSL)
            gt_h = gat_out[:, :, :].tensor

            # score loads for selection, spread across both HWDGE queues
            # sck[r*32+pp, h, c] = gat_out[r, 4hg+h, pp*32+c] (order-free)
            for h in range(HPC):
                nc.sync.dma_start(sck[:, h, :], AP(
                    tensor=gt_h, offset=hoff_sp + h * LSL,
                    ap=[[H * LSL, 4], [32, 32], [1, 32]]))
            # sc2[32h+pp, ff] = score[h, pp*256+ff]
            for h in range(HPC):
                nc.scalar.dma_start(sc2[32 * h:32 * h + 16, :], AP(
                    tensor=gt_h, offset=hoff_act + h * LSL,
                    ap=[[H * LSL, 4], [256, 4], [1, 256]]))

            # ---------- selection state ----------
            v4 = pp_.tile([1, HPC, 2], F32)
            sel = pp_.tile([128, 256], F32)
            idx4 = pp_.tile([16, HPC, 32], F32)
            nfound = pp_.tile([16, HPC], U32)
            idx16 = pp_.tile([128, HPC, 32], I16)
            kTs_t = pp_.tile([128, HPC, TOPK], F16)
            vselA = pp_.tile([128, HPC, 4, HD + 1], F16)
            nc.vector.memset(vselA[:, :, :, HD:HD + 1], 1.0)

            # ---------- pass B: q projection ----------
            for g in range(NG):
                sl = slice(g * GT, (g + 1) * GT)
                qp = psX.tile([128, 2, GT], F32, tag="psX", name="qp")
                for p2 in range(2):
                    for j in range(DC):
                        nc.tensor.matmul(
                            qp[:, p2, :], wqs[:, j, p2 * 128:(p2 + 1) * 128],
                            tokh[:, sl, j], start=(j == 0), stop=(j == DC - 1))
                    nc.vector.tensor_scalar_add(qT[:, p2, sl], qp[:, p2, :],
                                                bqs[:, p2:p2 + 1])

            def sel_phase():
                # engine-major emission: per-engine queues are in-order, so
                # grouping by engine keeps head h's chain from blocking
                # head h+1's independent instructions.
                for h in range(HPC):
                    nc.gpsimd.kth_largest(v4[0:1, h, :], sck[:, h, :],
                                          n_per_lane=32, k=510, quantile=KTH_Q)
                thps = []
                for h in range(HPC):
                    thp = psT.tile([128, 1], F32, tag="tp", name="thp")
                    nc.tensor.matmul(thp, b4[0:1, h, :], v4[0:1, h, 1:2],
                                     start=True, stop=True)
                    thps.append(thp)
                for h in range(HPC):
                    ssl = slice(32 * h, 32 * h + 16)
                    nc.vector.tensor_scalar(sel[ssl, :], sc2[ssl, :],
                                            thps[h][ssl, :], None, op0=OP.is_ge)
                    nc.vector.tensor_mul(sel[ssl, :], sel[ssl, :],
                                         iota1[ssl, :])
                    nc.vector.tensor_scalar_sub(sel[ssl, :], sel[ssl, :], 1.0)
                selhs = []
                for h in range(HPC):
                    selh = psm.tile([16, 256], F32, tag="selh", bufs=4)
                    # Act copy (partition-shifting) keeps Pool free for the
                    # kth/sparse_gather chain
                    nc.scalar.copy(selh, sel[32 * h:32 * h + 16, :])
                    selhs.append(selh)
                rps = {}

                def sel_tail(h):
                    nc.gpsimd.sparse_gather(
                        idx4[:, h, :], selhs[h], num_found=nfound[0:1, h:h + 1])
                    rp = psT.tile([128, 32], F32, tag="tp", name="rp")
                    nc.tensor.matmul(rp, r16, idx4[:, h, :],
                                     start=True, stop=True)
                    nc.vector.tensor_copy(idx16[:, h, :], rp)

                # head 0's tail first so its gather issues ahead of the rest
                sel_tail(0)
                return sel_tail

            def head_gather(h):
                toksel = pr.tile([128, 4, D], F16, tag="toksel",
                                 name="toksel", bufs=2)
                nc.gpsimd.dma_gather(
                    toksel, tok_lm[:, :], idx16[:, h, :], num_idxs=TOPK,
                    num_idxs_reg=TOPK, elem_size=D, elem_step=D)
                return toksel

            def head_xbar(h, toksel, jr=range(DC), tokB=None):
                # PE transposes: tokB[p, j, kt*128+k] = toksel[k, kt, j*128+p]
                if tokB is None:
                    tokB = pr.tile([128, DC, TOPK], F16, tag="tokB",
                                   name="tokB", bufs=2)
                for j in jr:
                    for half in range(2):
                        ttp = psT.tile([128, 2, 128], F16, tag="tp", name="ttp")
                        for kk in range(2):
                            kt = half * 2 + kk
                            nc.tensor.transpose(
                                ttp[:, kk, :],
                                toksel[:, kt, j * 128:(j + 1) * 128], ident16)
                        nc.vector.tensor_copy(
                            tokB[:, j, 256 * half:256 * (half + 1)],
                            ttp.rearrange("p a b -> p (a b)"))
                return tokB

            def head_proj_kv_a(h, tokB):
                kvp = psX.tile([128, TOPK], F32, tag="psX", name="kvp")
                for j in range(4):
                    nc.tensor.matmul(kvp, wkvs[:, j, 128 * h:128 * (h + 1)],
                                     tokB[:, j, :], start=(j == 0), stop=False)
                return kvp

            def head_proj_kv_b(h, kvp, tokB):
                h2 = h % 2
                # fused kv projection: host orders the wkv block so k lands on
                # rows [64h2, 64h2+64) (matching qT/kTs_t) and v on the other
                # 64 rows — all PSUM drains keep partition alignment for DVE.
                ksl = slice(64 * h2, 64 * h2 + 64)
                vsl = slice(64 - 64 * h2, 128 - 64 * h2)
                for j in range(4, DC):
                    nc.tensor.matmul(kvp, wkvs[:, j, 128 * h:128 * (h + 1)],
                                     tokB[:, j, :],
                                     start=False, stop=(j == DC - 1))
                nc.vector.tensor_copy(kTs_t[ksl, h, :], kvp[ksl, :])
                vTb = psm.tile([128, TOPK], F16, tag="vTb", bufs=2)
                nc.vector.tensor_scalar_add(vTb[vsl, :], kvp[vsl, :],
                                            bvs[vsl, h:h + 1])
                return vTb

            def head_proj_mm(h, tokB):
                kvp = head_proj_kv_a(h, tokB)
                return head_proj_kv_b(h, kvp, tokB)

            def head_proj_vsel(h, vTb):
                h2 = h % 2
                vsl = slice(64 - 64 * h2, 128 - 64 * h2)
                # PE transposes: vselA[p, h, kt, d] = v[d, kt*128+p]
                for kt in range(4):
                    vtp = psT.tile([128, HD], F16, tag="tp", name="vtp")
                    nc.tensor.transpose(vtp, vTb[vsl, kt * 128:(kt + 1) * 128],
                                        ident16[vsl, vsl])
                    nc.vector.tensor_copy(vselA[:, h, kt, 0:HD], vtp)

            def attn_logits(h, qc):
                p2, h2 = h // 2, h % 2
                expT = [None, None]
                for half in range(2):
                    lp = psX.tile([128, 2, GT], F32, tag="psX", name="lp")
                    for kk in range(2):
                        kt = half * 2 + kk
                        nc.tensor.matmul(
                            lp[:, kk, :],
                            kTs_t[64 * h2:64 * h2 + 64, h,
                                  kt * 128:(kt + 1) * 128],
                            qT[64 * h2:64 * h2 + 64, p2,
                               qc * 512:(qc + 1) * 512],
                            start=True, stop=True)
                    expT[half] = pr.tile([128, 2, GT], F16, tag="expT",
                                         name="expT", bufs=4)
                    nc.scalar.activation(
                        expT[half].rearrange("p a b -> p (a b)"),
                        lp.rearrange("p a b -> p (a b)"),
                        AF.Exp, scale=0.125)
                return expT

            def attn_av(h, qc, expT):
                avp = psT.tile([128, 4, HD + 1], F32, tag="tp", name="avp")
                for qs in range(4):
                    for kt in range(4):
                        nc.tensor.matmul(
                            avp[:, qs, :],
                            expT[kt // 2][:, kt % 2, qs * 128:(qs + 1) * 128],
                            vselA[:, h, kt, :],
                            start=(kt == 0), stop=(kt == 3))
                rcp = psm.tile([128, 4], F32, tag="rcp")
                nc.vector.reciprocal(
                    rcp, avp[:, :, HD:HD + 1].rearrange("p a b -> p (a b)"))
                for qs in range(4):
                    qt = qc * 4 + qs
                    nc.vector.tensor_scalar_mul(
                        obuf[:, qt, HD * h:HD * (h + 1)], avp[:, qs, 0:HD],
                        rcp[:, qs:qs + 1])

            sel_tail = sel_phase()
            toksel0 = head_gather(0)
            for h in range(1, HPC):
                sel_tail(h)
            tokB0 = head_xbar(0, toksel0)
            vTb0 = head_proj_mm(0, tokB0)
            # head 0's first logits go out before the v-side prep so the Act
            # engine's exp backbone starts as early as possible
            expT_cur = attn_logits(0, 0)
            head_proj_vsel(0, vTb0)

            # attention, software-pipelined one qc ahead: the next qc's
            # logits+exp are emitted before this qc's AV so exp runs
            # back-to-back on Act (the attention-phase bottleneck).
            tksl, tkb, tvtb = {}, {}, {}
            for h in range(HPC):
                for qc in range(8):
                    # next qc's logits first: the Act exp backbone must never
                    # wait behind prep work in the PE stream
                    if qc < 7:
                        expT_next = attn_logits(h, qc + 1)
                    elif h + 1 < HPC:
                        expT_next = attn_logits(h + 1, 0)
                    else:
                        expT_next = None
                    if h + 1 < HPC:
                        if qc == 0:
                            tksl[h + 1] = head_gather(h + 1)
                        elif qc <= 4:
                            jr = range(2 * (qc - 1), 2 * qc)
                            tkb[h + 1] = head_xbar(
                                h + 1, tksl[h + 1], jr=jr,
                                tokB=tkb.get(h + 1) if qc > 1 else None)
                        elif qc == 5:
                            tvtb[h + 1] = head_proj_kv_a(h + 1, tkb[h + 1])
                        elif qc == 6:
                            tvtb[h + 1] = head_proj_kv_b(
                                h + 1, tvtb[h + 1], tkb[h + 1])
                        elif qc == 7:
                            head_proj_vsel(h + 1, tvtb[h + 1])
                    attn_av(h, qc, expT_cur)
                    expT_cur = expT_next
                    if h == HPC - 1:
                        # sync queue: idle during attention, and splitting the
                        # last block halves the post-compute drain.
                        osl = out[qc * 512:(qc + 1) * 512, :].rearrange(
                            "(q p) n -> p q n", p=128)
                        if qc < 7:
                            nc.sync.dma_start(osl, obuf[:, qc * 4:(qc + 1) * 4, :])
                        else:
                            nc.sync.dma_start(osl[:, 0:2, :],
                                              obuf[:, qc * 4:qc * 4 + 2, :])
                            nc.sync.dma_start(osl[:, 2:4, :],
                                              obuf[:, qc * 4 + 2:qc * 4 + 4, :])

            # dbg_scores[h, r*1024 + t] = gat_out[r, 4hg+h, t] — test-harness
            # output only; emitted last so it never touches the critical path.
            nc.gpsimd.dma_start(
                dbg_scores[:, :].rearrange("h (r t) -> h r t", r=4),
                AP(tensor=gt_h, offset=hoff,
                   ap=[[LSL, HPC], [H * LSL, 4], [1, LSL]]))

    nc.compile()
    return nc


_NC = None


def _get_nc():
    global _NC
    if _NC is None:
        _NC = build_nc()
    return _NC


def make_in_maps(**inputs):
    t = {k: np.ascontiguousarray(np.asarray(v, dtype=np.float32))
         for k, v in inputs.items()}
    wi1h = t["Wi1"].astype(np.float16)
    wi1l = (t["Wi1"] - wi1h.astype(np.float32)).astype(np.float16)
    in_maps = []
    for c in range(8):
        b, hg = c // 4, c % 4
        cs = COLS * hg
        wkv_c = np.empty((D, 2 * COLS), dtype=np.float16)
        for hh in range(HPC):
            # k on rows [64*(hh%2), +64), v on the complementary 64 rows
            ko, vo = (0, 64) if hh % 2 == 0 else (64, 0)
            wkv_c[:, 128 * hh + ko:128 * hh + ko + 64] = \
                t["Wk"][:, cs + 64 * hh:cs + 64 * hh + 64]
            wkv_c[:, 128 * hh + vo:128 * hh + vo + 64] = \
                t["Wv"][:, cs + 64 * hh:cs + 64 * hh + 64]
        tokT = np.ascontiguousarray(t["tokens"][b].T)   # [D, L]
        th = tokT.astype(np.float16)
        # rotate so this core's MLP slice sits at rows 0:1024
        th_rot = np.roll(th, -LSL * hg, axis=1)
        sl = slice(LSL * hg, LSL * (hg + 1))
        tl = (tokT[:, sl] - th[:, sl].astype(np.float32)).astype(np.float16)
        # gather-friendly [128, L, 8]: th_g[p, l, c] = th[c*128+p, l]
        th_rot = np.ascontiguousarray(
            th_rot.reshape(DC, 128, L).transpose(1, 2, 0))
        tl = np.ascontiguousarray(tl.reshape(DC, 128, LSL).transpose(1, 2, 0))
        pp = np.arange(128) % 32
        iota_np = np.where(
            pp[:, None] < 16,
            (pp[:, None] * 256 + np.arange(256)[None, :]).astype(np.float64)
            + 1.0,
            -1e9).astype(np.float32)
        in_maps.append({
            "th_t": th_rot,
            "tl_t": tl,
            "tok_lm": np.ascontiguousarray(t["tokens"][b].astype(np.float16)),
            "iota_r": np.ascontiguousarray(iota_np),
            "wq": np.ascontiguousarray(t["Wq"][:, cs:cs + COLS]).astype(np.float16),
            "wkv": wkv_c,
            "bqt": np.ascontiguousarray(t["bq"][cs:cs + COLS].reshape(2, 128).T),
            "bvt": np.ascontiguousarray(
                np.tile(t["bv"][cs:cs + COLS].reshape(4, 64).T, (2, 1))),
            "wi1h": wi1h,
            "wi1l": wi1l,
            "wi2f": t["Wi2"],
            "bi1t": np.ascontiguousarray(t["bi1"].reshape(2, 128).T),
            "bi2": np.ascontiguousarray(t["bi2"].reshape(H, 1)),
        })
    return in_maps


def kernel(**inputs) -> np.ndarray:
    nc = _get_nc()
    in_maps = make_in_maps(**inputs)
    res = run_bass_kernel_spmd(nc, in_maps, core_ids=list(range(8)))
    out = np.empty((B, L, D), dtype=np.float32)
    for c in range(8):
        b, hg = c // 4, c % 4
        o = res.results[c]["out"].astype(np.float32)
        out[b, :, COLS * hg:COLS * (hg + 1)] = np.roll(o, LSL * hg, axis=0)
    return out


# revision 59
# speedup vs baseline: 1.0253x; 1.0127x over previous
"""DynamicSparseAttention Trainium2 kernel (v4).

Shards B=2 x H=16 across 8 NeuronCores: core c handles batch c//4 and the
4 heads [4*(c%4), 4*(c%4)+4).  Self-contained: all shapes hardcoded.

v4 design (vs v3):
- importance MLP is L-sharded across the 4 cores of each batch: each core
  computes exact 3-term-fp16 scores for its 1024-token slice x all 16 heads,
  then an AllGather over replica groups [[0..3],[4..7]] shares them.  One
  dynamic-offset DMA (partition_id-based) extracts this core's 4-head block.
  Tokens arrive pre-rotated per core so the local slice is rows 0:1024.
- gathered-token transposes use the DMA XBAR (dma transpose, 16x128 tiles)
  instead of PE transposes + DVE copies.
- AV is computed directly in [query, head_dim] orientation (lhsT = expT),
  eliminating the output transposes; normalization reads PSUM directly.
- MLP layer 2 runs in fp32 on the PE (exact; removes the fp16 split).
"""
import numpy as np

import concourse.bass as bass
import concourse.mybir as mybir
import concourse.tile as tile
from concourse import bacc
from concourse.ap import AP
from concourse.bass_utils import run_bass_kernel_spmd

F32 = mybir.dt.float32
F16 = mybir.dt.float16
I16 = mybir.dt.int16
U32 = mybir.dt.uint32
AF = mybir.ActivationFunctionType
OP = mybir.AluOpType

B, L, D = 2, 4096, 1024
H, HD, TOPK = 16, 64, 512
HIDDEN = 256
HPC = 4                # heads per core
COLS = HPC * HD        # 256 output cols per core
NG = 8                 # token groups
GT = 512               # tokens per group
DC = 8                 # 128-row chunks of D
LSL = L // 4           # local MLP token slice
KTH_Q = 1.0 - 510.5 / 4095.0   # k_adj=510 -> out[0,1] = 512th largest
RG = [[0, 1, 2, 3], [4, 5, 6, 7]]


def build_nc():
    nc = bacc.Bacc("TRN2", target_bir_lowering=False, num_devices=8)

    th_t = nc.dram_tensor("th_t", [128, L, DC], F16, kind="ExternalInput")
    tl_t = nc.dram_tensor("tl_t", [128, LSL, DC], F16, kind="ExternalInput")
    tok_lm = nc.dram_tensor("tok_lm", [L, D], F16, kind="ExternalInput")
    iota_r = nc.dram_tensor("iota_r", [128, 256], F32, kind="ExternalInput")
    wq = nc.dram_tensor("wq", [D, COLS], F16, kind="ExternalInput")
    wkv = nc.dram_tensor("wkv", [D, 2 * COLS], F16, kind="ExternalInput")
    bqt = nc.dram_tensor("bqt", [128, 2], F32, kind="ExternalInput")
    bvt = nc.dram_tensor("bvt", [128, HPC], F32, kind="ExternalInput")
    wi1h = nc.dram_tensor("wi1h", [D, HIDDEN], F16, kind="ExternalInput")
    wi1l = nc.dram_tensor("wi1l", [D, HIDDEN], F16, kind="ExternalInput")
    wi2f = nc.dram_tensor("wi2f", [HIDDEN, H], F32, kind="ExternalInput")
    bi1t = nc.dram_tensor("bi1t", [128, 2], F32, kind="ExternalInput")
    bi2 = nc.dram_tensor("bi2", [H, 1], F32, kind="ExternalInput")

    gat_in = nc.dram_tensor("gat_in", [H, LSL], F32, kind="Internal")
    gat_out = nc.dram_tensor("gat_out", [4, H, LSL], F32, kind="Internal")

    out = nc.dram_tensor("out", [L, COLS], F16, kind="ExternalOutput")
    dbg_scores = nc.dram_tensor("dbg_scores", [HPC, L], F32, kind="ExternalOutput")

    # ---- constants ----
    ident16_d = nc.inline_tensor(np.eye(128, dtype=np.float16), name="ident16")
    r16_np = (np.arange(16)[:, None] == (np.arange(128)[None, :] % 16)).astype(np.float32)
    r16_d = nc.inline_tensor(r16_np, name="r16")
    b4_np = (np.arange(128)[None, None, :] // 32
             == np.arange(4)[None, :, None]).astype(np.float32)
    b4_d = nc.inline_tensor(b4_np, name="b4")  # [1, 4, 128]

    with tile.TileContext(nc) as tc:
        with (
            tc.tile_pool(name="persist", bufs=1) as pp_,
            tc.tile_pool(name="ring", bufs=2) as pr,
            tc.tile_pool(name="small", bufs=2) as psm,
            tc.tile_pool(name="psX", bufs=3, space="PSUM") as psX,
            tc.tile_pool(name="psT", bufs=2, space="PSUM") as psT,
        ):
            # ---------- persistent ----------
            tokh = pp_.tile([128, L, DC], F16)       # 64KB/part
            qT = pp_.tile([128, 2, L], F16)          # 16KB/part
            sc2 = pp_.tile([128, 256], F32)
            # rows 16:32 of each 32-row head block are never DMA'd; zero them
            # once so the is_ge mask reads defined data (iota masks them out).
            nc.gpsimd.memset(sc2, 0.0)
            sck = pp_.tile([128, HPC, 32], F32)
            obuf = pp_.tile([128, 32, COLS], F16)    # 16KB/part

            # ---------- input DMAs, ordered for the MLP critical path ------
            # sync (SP) queue: MLP tokens, then (blocking) imp stages
            # scalar (Act) queue: weights, then the q-proj token stream
            # PE warm-up: the cost model clocks the PE up only after ~3us of
            # activity (LOW->MID->FULL). Run a throwaway accumulation chain on
            # the identity tile (first DMA, 32KB) so the ramp completes while
            # the real inputs are still streaming in.
            ident16 = pp_.tile([128, 128], F16)
            nc.sync.dma_start(ident16, ident16_d[:, :])
            warm = psT.tile([128, 128], F32, tag="tp", name="warm")
            NWARM = 90
            for i in range(NWARM):
                nc.tensor.matmul(warm, ident16, ident16,
                                 start=(i == 0), stop=(i == NWARM - 1))
            # preload the Act function tables (Gelu/Exp/Identity) while the
            # engine is idle so no 1.3us table load lands on the critical path
            actwarm = psm.tile([1, 2], F16, tag="actwarm")
            nc.scalar.activation(actwarm[0:1, 0:1], ident16[0:1, 0:1],
                                 AF.Gelu, scale=1.0)
            nc.scalar.activation(actwarm[0:1, 1:2], ident16[0:1, 0:1],
                                 AF.Exp, scale=1.0)
            nc.scalar.copy(actwarm[0:1, 0:1], ident16[0:1, 0:1])

            tokls = []
            wi1hs = pp_.tile([128, DC, HIDDEN], F16)
            wi1ls = pp_.tile([128, DC, HIDDEN], F16)
            for g in range(2):
                sl = slice(g * GT, (g + 1) * GT)
                nc.sync.dma_start(tokh[:, sl, :], th_t[:, sl, :])
                w, wsrc = (wi1hs, wi1h) if g == 0 else (wi1ls, wi1l)
                nc.scalar.dma_start(w, wsrc.rearrange("(c p) n -> p c n", p=128))
                tokl = pr.tile([128, GT, DC], F16, tag="tokl", name="tokl")
                nc.sync.dma_start(tokl, tl_t[:, sl, :])
                tokls.append(tokl)
            wi2fs = pp_.tile([128, 2, H], F32)
            nc.scalar.dma_start(wi2fs, wi2f.rearrange("(c p) n -> p c n", p=128))
            bi1s = pp_.tile([128, 2], F32)
            nc.sync.dma_start(bi1s, bi1t[:, :])
            bi2s = pp_.tile([H, 1], F32)
            nc.sync.dma_start(bi2s, bi2[:, :])
            bqs = pp_.tile([128, 2], F32)
            nc.sync.dma_start(bqs, bqt[:, :])
            bvs = pp_.tile([128, HPC], F32)
            nc.sync.dma_start(bvs, bvt[:, :])
            iota1 = pp_.tile([128, 256], F32)
            nc.sync.dma_start(iota1, iota_r[:, :])
            r16 = pp_.tile([16, 128], F32)
            nc.sync.dma_start(r16, r16_d[:, :])
            b4 = pp_.tile([1, 4, 128], F32)
            nc.sync.dma_start(b4, b4_d[:, :, :])
            wqs = pp_.tile([128, DC, COLS], F16)
            nc.scalar.dma_start(wqs, wq.rearrange("(c p) n -> p c n", p=128))
            wkvs = pp_.tile([128, DC, 2 * COLS], F16)
            nc.scalar.dma_start(wkvs, wkv.rearrange("(c p) n -> p c n", p=128))

            # ---------- phase A: local-slice importance MLP ----------
            hpss = []
            for g in range(2):
                sl = slice(g * GT, (g + 1) * GT)
                hps = psX.tile([128, 2, GT], F32, tag="psX")
                hpss.append(hps)
                # term-outer order: the first 16 matmuls only need th + wi1h,
                # so compute starts before tl/wi1l arrive.
                terms = ((wi1hs, tokh[:, sl, :]), (wi1hs, tokls[g]),
                         (wi1ls, tokh[:, sl, :]))
                for ti, (w, a) in enumerate(terms):
                    for ht in range(2):
                        hsl = slice(ht * 128, (ht + 1) * 128)
                        for j in range(DC):
                            nc.tensor.matmul(
                                hps[:, ht, :], w[:, j, hsl], a[:, :, j],
                                start=(ti == 0 and j == 0),
                                stop=(ti == 2 and j == DC - 1))
            for g in range(2):
                sl = slice(g * GT, (g + 1) * GT)
                gtmp = pr.tile([128, 2, GT], F32, tag="gtmp", name="gtmp")
                for ht in range(2):
                    nc.scalar.activation(gtmp[:, ht, :], hpss[g][:, ht, :],
                                         AF.Gelu, bias=bi1s[:, ht:ht + 1],
                                         scale=1.0)
                ips = psX.tile([H, GT], F32, tag="psX", name="ips")
                for kc in range(2):
                    nc.tensor.matmul(ips, wi2fs[:, kc, :], gtmp[:, kc, :],
                                     start=(kc == 0), stop=(kc == 1))
                imp_c = psm.tile([H, GT], F32, tag="imp_c")
                nc.vector.tensor_scalar_add(imp_c, ips, bi2s)
                nc.sync.dma_start(gat_in[:, sl], imp_c)

            # ---------- score all-gather across the 4 cores of this batch ----
            nc.gpsimd.collective_compute(
                "AllGather", mybir.AluOpType.bypass, replica_groups=RG,
                ins=[gat_in[:, :].opt()], outs=[gat_out[:, :, :].opt()])

            # scheduler fence: without it the (dep-free) token-stream DMAs
            # below get scheduled ahead of the imp DMAs and delay the
            # collective behind 18us of streaming at the DMA-engine FIFO.
            tc.no_sync_barrier()

            # q-proj token stream: emitted after the imp stages so its DMA
            # transfers queue behind them at the (FIFO) DMA engines.
            for g in range(2, NG):
                sl = slice(g * GT, (g + 1) * GT)
                nc.sync.dma_start(tokh[:, sl, :], th_t[:, sl, :])

            # dynamic offsets (elements) of this core's 4-head block
            pid_sp = nc.sync.partition_id()
            hoff_sp = (pid_sp % 4) * (HPC * LSL)
            pid_act = nc.scalar.partition_id()
            hoff_act = (pid_act % 4) * (HPC * LSL)
            pid = nc.gpsimd.partition_id()
            hoff = (pid % 4) * (HPC * LSL)
            gt_h = gat_out[:, :, :].tensor

            # score loads for selection, spread across both HWDGE queues
            # sck[r*32+pp, h, c] = gat_out[r, 4hg+h, pp*32+c] (order-free)
            for h in range(HPC):
                nc.sync.dma_start(sck[:, h, :], AP(
                    tensor=gt_h, offset=hoff_sp + h * LSL,
                    ap=[[H * LSL, 4], [32, 32], [1, 32]]))
            # sc2[32h+pp, ff] = score[h, pp*256+ff]
            for h in range(HPC):
                nc.scalar.dma_start(sc2[32 * h:32 * h + 16, :], AP(
                    tensor=gt_h, offset=hoff_act + h * LSL,
                    ap=[[H * LSL, 4], [256, 4], [1, 256]]))

            # ---------- selection state ----------
            v4 = pp_.tile([1, HPC, 2], F32)
            sel = pp_.tile([128, 256], F32)
            idx4 = pp_.tile([16, HPC, 32], F32)
            nfound = pp_.tile([16, HPC], U32)
            idx16 = pp_.tile([128, HPC, 32], I16)
            kTs_t = pp_.tile([128, HPC, TOPK], F16)
            vselA = pp_.tile([128, HPC, 4, HD + 1], F16)
            nc.vector.memset(vselA[:, :, :, HD:HD + 1], 1.0)

            # ---------- pass B: q projection ----------
            for g in range(NG):
                sl = slice(g * GT, (g + 1) * GT)
                qp = psX.tile([128, 2, GT], F32, tag="psX", name="qp")
                for p2 in range(2):
                    for j in range(DC):
                        nc.tensor.matmul(
                            qp[:, p2, :], wqs[:, j, p2 * 128:(p2 + 1) * 128],
                            tokh[:, sl, j], start=(j == 0), stop=(j == DC - 1))
                    nc.vector.tensor_scalar_add(qT[:, p2, sl], qp[:, p2, :],
                                                bqs[:, p2:p2 + 1])

            def sel_phase():
                # engine-major emission: per-engine queues are in-order, so
                # grouping by engine keeps head h's chain from blocking
                # head h+1's independent instructions.
                for h in range(HPC):
                    nc.gpsimd.kth_largest(v4[0:1, h, :], sck[:, h, :],
                                          n_per_lane=32, k=510, quantile=KTH_Q)
                thps = []
                for h in range(HPC):
                    thp = psT.tile([128, 1], F32, tag="tp", name="thp")
                    nc.tensor.matmul(thp, b4[0:1, h, :], v4[0:1, h, 1:2],
                                     start=True, stop=True)
                    thps.append(thp)
                for h in range(HPC):
                    ssl = slice(32 * h, 32 * h + 16)
                    nc.vector.tensor_scalar(sel[ssl, :], sc2[ssl, :],
                                            thps[h][ssl, :], None, op0=OP.is_ge)
                    nc.vector.tensor_mul(sel[ssl, :], sel[ssl, :],
                                         iota1[ssl, :])
                    nc.vector.tensor_scalar_sub(sel[ssl, :], sel[ssl, :], 1.0)
                selhs = []
                for h in range(HPC):
                    selh = psm.tile([16, 256], F32, tag="selh", bufs=4)
                    # Act copy (partition-shifting) keeps Pool free for the
                    # kth/sparse_gather chain
                    nc.scalar.copy(selh, sel[32 * h:32 * h + 16, :])
                    selhs.append(selh)
                rps = {}

                def sel_tail(h):
                    nc.gpsimd.sparse_gather(
                        idx4[:, h, :], selhs[h], num_found=nfound[0:1, h:h + 1])
                    rp = psT.tile([128, 32], F32, tag="tp", name="rp")
                    nc.tensor.matmul(rp, r16, idx4[:, h, :],
                                     start=True, stop=True)
                    nc.vector.tensor_copy(idx16[:, h, :], rp)

                # head 0's tail first so its gather issues ahead of the rest
                sel_tail(0)
                return sel_tail

            def head_gather(h):
                toksel = pr.tile([128, 4, D], F16, tag="toksel",
                                 name="toksel", bufs=2)
                nc.gpsimd.dma_gather(
                    toksel, tok_lm[:, :], idx16[:, h, :], num_idxs=TOPK,
                    num_idxs_reg=TOPK, elem_size=D, elem_step=D)
                return toksel

            def head_xbar(h, toksel, jr=range(DC), tokB=None):
                # PE transposes: tokB[p, j, kt*128+k] = toksel[k, kt, j*128+p]
                if tokB is None:
                    tokB = pr.tile([128, DC, TOPK], F16, tag="tokB",
                                   name="tokB", bufs=2)
                for j in jr:
                    for half in range(2):
                        ttp = psT.tile([128, 2, 128], F16, tag="tp", name="ttp")
                        for kk in range(2):
                            kt = half * 2 + kk
                            nc.tensor.transpose(
                                ttp[:, kk, :],
                                toksel[:, kt, j * 128:(j + 1) * 128], ident16)
                        nc.vector.tensor_copy(
                            tokB[:, j, 256 * half:256 * (half + 1)],
                            ttp.rearrange("p a b -> p (a b)"))
                return tokB

            def head_proj_kv_a(h, tokB):
                kvp = psX.tile([128, TOPK], F32, tag="psX", name="kvp")
                for j in range(4):
                    nc.tensor.matmul(kvp, wkvs[:, j, 128 * h:128 * (h + 1)],
                                     tokB[:, j, :], start=(j == 0), stop=False)
                return kvp

            def head_proj_kv_b(h, kvp, tokB):
                h2 = h % 2
                # fused kv projection: host orders the wkv block so k lands on
                # rows [64h2, 64h2+64) (matching qT/kTs_t) and v on the other
                # 64 rows — all PSUM drains keep partition alignment for DVE.
                ksl = slice(64 * h2, 64 * h2 + 64)
                vsl = slice(64 - 64 * h2, 128 - 64 * h2)
                for j in range(4, DC):
                    nc.tensor.matmul(kvp, wkvs[:, j, 128 * h:128 * (h + 1)],
                                     tokB[:, j, :],
                                     start=False, stop=(j == DC - 1))
                nc.vector.tensor_copy(kTs_t[ksl, h, :], kvp[ksl, :])
                vTb = psm.tile([128, TOPK], F16, tag="vTb", bufs=2)
                nc.vector.tensor_scalar_add(vTb[vsl, :], kvp[vsl, :],
                                            bvs[vsl, h:h + 1])
                return vTb

            def head_proj_mm(h, tokB):
                kvp = head_proj_kv_a(h, tokB)
                return head_proj_kv_b(h, kvp, tokB)

            def head_proj_vsel(h, vTb):
                h2 = h % 2
                vsl = slice(64 - 64 * h2, 128 - 64 * h2)
                # PE transposes: vselA[p, h, kt, d] = v[d, kt*128+p]
                for kt in range(4):
                    vtp = psT.tile([128, HD], F16, tag="tp", name="vtp")
                    nc.tensor.transpose(vtp, vTb[vsl, kt * 128:(kt + 1) * 128],
                                        ident16[vsl, vsl])
                    nc.vector.tensor_copy(vselA[:, h, kt, 0:HD], vtp)

            def attn_logits(h, qc):
                p2, h2 = h // 2, h % 2
                expT = [None, None]
                for half in range(2):
                    lp = psX.tile([128, 2, GT], F32, tag="psX", name="lp")
                    for kk in range(2):
                        kt = half * 2 + kk
                        nc.tensor.matmul(
                            lp[:, kk, :],
                            kTs_t[64 * h2:64 * h2 + 64, h,
                                  kt * 128:(kt + 1) * 128],
                            qT[64 * h2:64 * h2 + 64, p2,
                               qc * 512:(qc + 1) * 512],
                            start=True, stop=True)
                    expT[half] = pr.tile([128, 2, GT], F16, tag="expT",
                                         name="expT", bufs=6)
                    nc.scalar.activation(
                        expT[half].rearrange("p a b -> p (a b)"),
                        lp.rearrange("p a b -> p (a b)"),
                        AF.Exp, scale=0.125)
                return expT

            def attn_av(h, qc, expT):
                avp = psT.tile([128, 4, HD + 1], F32, tag="tp", name="avp")
                for qs in range(4):
                    for kt in range(4):
                        nc.tensor.matmul(
                            avp[:, qs, :],
                            expT[kt // 2][:, kt % 2, qs * 128:(qs + 1) * 128],
                            vselA[:, h, kt, :],
                            start=(kt == 0), stop=(kt == 3))
                rcp = psm.tile([128, 4], F32, tag="rcp")
                nc.vector.reciprocal(
                    rcp, avp[:, :, HD:HD + 1].rearrange("p a b -> p (a b)"))
                for qs in range(4):
                    qt = qc * 4 + qs
                    nc.vector.tensor_scalar_mul(
                        obuf[:, qt, HD * h:HD * (h + 1)], avp[:, qs, 0:HD],
                        rcp[:, qs:qs + 1])

            sel_tail = sel_phase()
            toksel0 = head_gather(0)
            for h in range(1, HPC):
                sel_tail(h)
            tokB0 = head_xbar(0, toksel0)
            vTb0 = head_proj_mm(0, tokB0)
            # head 0's first logits go out before the v-side prep so the Act
            # engine's exp backbone starts as early as possible
            expT_cur = attn_logits(0, 0)
            head_proj_vsel(0, vTb0)

            # attention, software-pipelined one qc ahead: the next qc's
            # logits+exp are emitted before this qc's AV so exp runs
            # back-to-back on Act (the attention-phase bottleneck).
            tksl, tkb, tvtb = {}, {}, {}
            for h in range(HPC):
                for qc in range(8):
                    # next qc's logits first: the Act exp backbone must never
                    # wait behind prep work in the PE stream
                    if qc < 7:
                        expT_next = attn_logits(h, qc + 1)
                    elif h + 1 < HPC:
                        expT_next = attn_logits(h + 1, 0)
                    else:
                        expT_next = None
                    if h + 1 < HPC:
                        if qc == 0:
                            tksl[h + 1] = head_gather(h + 1)
                        elif qc <= 4:
                            jr = range(2 * (qc - 1), 2 * qc)
                            tkb[h + 1] = head_xbar(
                                h + 1, tksl[h + 1], jr=jr,
                                tokB=tkb.get(h + 1) if qc > 1 else None)
                        elif qc == 5:
                            tvtb[h + 1] = head_proj_kv_a(h + 1, tkb[h + 1])
                        elif qc == 6:
                            tvtb[h + 1] = head_proj_kv_b(
                                h + 1, tvtb[h + 1], tkb[h + 1])
                        elif qc == 7:
                            head_proj_vsel(h + 1, tvtb[h + 1])
                    attn_av(h, qc, expT_cur)
                    expT_cur = expT_next
                    if h == HPC - 1:
                        # sync queue: idle during attention, and splitting the
                        # last block halves the post-compute drain.
                        osl = out[qc * 512:(qc + 1) * 512, :].rearrange(
                            "(q p) n -> p q n", p=128)
                        if qc < 7:
                            nc.sync.dma_start(osl, obuf[:, qc * 4:(qc + 1) * 4, :])
                        else:
                            nc.sync.dma_start(osl[:, 0:2, :],
                                              obuf[:, qc * 4:qc * 4 + 2, :])
                            nc.sync.dma_start(osl[:, 2:4, :],
                                              obuf[:, qc * 4 + 2:qc * 4 + 4, :])

            # dbg_scores[h, r*1024 + t] = gat_out[r, 4hg+h, t] — test-harness
            # output only; emitted last so it never touches the critical path.
            nc.gpsimd.dma_start(
                dbg_scores[:, :].rearrange("h (r t) -> h r t", r=4),
                AP(tensor=gt_h, offset=hoff,
                   ap=[[LSL, HPC], [H * LSL, 4], [1, LSL]]))

    nc.compile()
    return nc


_NC = None


def _get_nc():
    global _NC
    if _NC is None:
        _NC = build_nc()
    return _NC


def make_in_maps(**inputs):
    t = {k: np.ascontiguousarray(np.asarray(v, dtype=np.float32))
         for k, v in inputs.items()}
    wi1h = t["Wi1"].astype(np.float16)
    wi1l = (t["Wi1"] - wi1h.astype(np.float32)).astype(np.float16)
    in_maps = []
    for c in range(8):
        b, hg = c // 4, c % 4
        cs = COLS * hg
        wkv_c = np.empty((D, 2 * COLS), dtype=np.float16)
        for hh in range(HPC):
            # k on rows [64*(hh%2), +64), v on the complementary 64 rows
            ko, vo = (0, 64) if hh % 2 == 0 else (64, 0)
            wkv_c[:, 128 * hh + ko:128 * hh + ko + 64] = \
                t["Wk"][:, cs + 64 * hh:cs + 64 * hh + 64]
            wkv_c[:, 128 * hh + vo:128 * hh + vo + 64] = \
                t["Wv"][:, cs + 64 * hh:cs + 64 * hh + 64]
        tokT = np.ascontiguousarray(t["tokens"][b].T)   # [D, L]
        th = tokT.astype(np.float16)
        # rotate so this core's MLP slice sits at rows 0:1024
        th_rot = np.roll(th, -LSL * hg, axis=1)
        sl = slice(LSL * hg, LSL * (hg + 1))
        tl = (tokT[:, sl] - th[:, sl].astype(np.float32)).astype(np.float16)
        # gather-friendly [128, L, 8]: th_g[p, l, c] = th[c*128+p, l]
        th_rot = np.ascontiguousarray(
            th_rot.reshape(DC, 128, L).transpose(1, 2, 0))
        tl = np.ascontiguousarray(tl.reshape(DC, 128, LSL).transpose(1, 2, 0))
        pp = np.arange(128) % 32
        iota_np = np.where(
            pp[:, None] < 16,
            (pp[:, None] * 256 + np.arange(256)[None, :]).astype(np.float64)
            + 1.0,
            -1e9).astype(np.float32)
        in_maps.append({
            "th_t": th_rot,
            "tl_t": tl,
            "tok_lm": np.ascontiguousarray(t["tokens"][b].astype(np.float16)),
            "iota_r": np.ascontiguousarray(iota_np),
            "wq": np.ascontiguousarray(t["Wq"][:, cs:cs + COLS]).astype(np.float16),
            "wkv": wkv_c,
            "bqt": np.ascontiguousarray(t["bq"][cs:cs + COLS].reshape(2, 128).T),
            "bvt": np.ascontiguousarray(
                np.tile(t["bv"][cs:cs + COLS].reshape(4, 64).T, (2, 1))),
            "wi1h": wi1h,
            "wi1l": wi1l,
            "wi2f": t["Wi2"],
            "bi1t": np.ascontiguousarray(t["bi1"].reshape(2, 128).T),
            "bi2": np.ascontiguousarray(t["bi2"].reshape(H, 1)),
        })
    return in_maps


def kernel(**inputs) -> np.ndarray:
    nc = _get_nc()
    in_maps = make_in_maps(**inputs)
    res = run_bass_kernel_spmd(nc, in_maps, core_ids=list(range(8)))
    out = np.empty((B, L, D), dtype=np.float32)
    for c in range(8):
        b, hg = c // 4, c % 4
        o = res.results[c]["out"].astype(np.float32)
        out[b, :, COLS * hg:COLS * (hg + 1)] = np.roll(o, LSL * hg, axis=0)
    return out
